# revision 13
# baseline (speedup 1.0000x reference)
"""AttentionBlock TRN2 kernel: data-parallel over batch (1 sample per core).

Everything on-chip is channel-major [c on partitions, t=h*w on free dim];
host pre-transposes x / weights and post-transposes the output.
"""

import numpy as np
import ml_dtypes

import concourse.bass as bass
from concourse import bacc
import concourse.mybir as mybir
import concourse.tile as tile
from concourse.bass_utils import run_bass_kernel_spmd

F32 = mybir.dt.float32
F32R = mybir.dt.float32r
BF16 = mybir.dt.bfloat16
FP16 = mybir.dt.float16

N_CORES = 8
T = 1024          # tokens = h*w = 32*32
C = 512           # channels
H = 8             # heads
D = 64            # head dim
G = 32            # groups
GSIZE = C // G    # 16 channels per group
EPS = 1e-5
CSUB = C // 128   # 4 channel subtiles
TSUB = T // 128   # 8 token subtiles
NELEM_GROUP = float(GSIZE)  # channel stats are already per-t means

_CACHE = {}


def _groupnorm(nc, tc, pools, xcm, gscale, goff, a_agg, a_map, out_ab, dbg=None):
    """Compute per-channel affine (a, b) implementing GroupNorm on channel-major
    xcm [128, CSUB, T].  a = rstd_g*scale_c, b = off_c - mean_g*rstd_g*scale_c.
    Writes them into out_ab ([128, CSUB] each).  Returns (a, b)."""
    sb, ps = pools["sb_small"], pools["psum"]
    a_t, b_t = out_ab

    stats = sb.tile([128, CSUB, 2], F32, tag="gn_stats")
    seg = sb.tile([128, 2, 6], F32, tag="gn_seg")
    for i in range(CSUB):
        for j in range(2):
            nc.vector.bn_stats(out=seg[:, j, :], in_=xcm[:, i, j * 512:(j + 1) * 512])
        nc.vector.bn_aggr(out=stats[:, i, :], in_=seg[:, :, :])
    # stats[:, :, 0]=mean_c, [:, :, 1]=var_c -> convert var to E[x^2]
    msq = sb.tile([128, CSUB], F32, tag="gn_msq")
    nc.vector.tensor_tensor(msq[:, :], stats[:, :, 0], stats[:, :, 0],
                            mybir.AluOpType.mult)
    nc.vector.tensor_tensor(stats[:, :, 1], stats[:, :, 1], msq[:, :],
                            mybir.AluOpType.add)
    # group aggregation: out[g_local, (i, s)] = sum over 16-ch groups
    gst_ps = ps.tile([128, 2, 512], F32, tag="acc")  # reuse accum psum slot
    gps = gst_ps[0:8, 0, 0:2 * CSUB]
    nc.tensor.matmul(gps, lhsT=a_agg[:, :], rhs=stats[:, :, :], start=True, stop=True)
    gs = sb.tile([8, CSUB, 2], F32, tag="gn_gs")
    nc.vector.tensor_scalar_mul(gs[:, :, :], gps, 1.0 / NELEM_GROUP)
    # var_g = msq_g - mean_g^2 ; rstd = exp(-0.5*ln(var+eps))
    vg = sb.tile([8, CSUB], F32, tag="gn_vg")
    nc.vector.tensor_tensor(vg[:, :], gs[:, :, 0], gs[:, :, 0], mybir.AluOpType.mult)
    nc.vector.tensor_tensor(vg[:, :], gs[:, :, 1], vg[:, :],
                            mybir.AluOpType.subtract)
    eps_t = sb.tile([8, 1], F32, tag="gn_eps")
    nc.vector.memset(eps_t[:, :], float(EPS))
    nc.scalar.activation(out=vg[:, :], in_=vg[:, :],
                         func=mybir.ActivationFunctionType.Ln, bias=eps_t[:, :],
                         scale=1.0)
    rstd = sb.tile([8, CSUB, 2], F32, tag="gn_rstd")
    nc.scalar.activation(out=rstd[:, :, 0], in_=vg[:, :],
                         func=mybir.ActivationFunctionType.Exp, scale=-0.5)
    nc.vector.tensor_tensor(rstd[:, :, 1], gs[:, :, 0], rstd[:, :, 0],
                            mybir.AluOpType.mult)
    # map back to channels: ab_ps[p, (i, s)] = (rstd_c, mean*rstd_c)
    ab_big = ps.tile([128, 2, 512], F32, tag="acc")
    ab_ps = ab_big[:, 0, 0:2 * CSUB]
    nc.tensor.matmul(ab_ps, lhsT=a_map[:, :], rhs=rstd[:, :, :], start=True, stop=True)
    ab_v = ab_ps.rearrange("p (i s) -> p i s", s=2)
    nc.vector.tensor_tensor(a_t[:, :], ab_v[:, :, 0], gscale[:, :],
                            mybir.AluOpType.mult)
    tmpb = sb.tile([128, CSUB], F32, tag="gn_tmpb")
    nc.vector.tensor_tensor(tmpb[:, :], ab_v[:, :, 1], gscale[:, :],
                            mybir.AluOpType.mult)
    nc.vector.tensor_tensor(b_t[:, :], goff[:, :], tmpb[:, :],
                            mybir.AluOpType.subtract)
    if dbg is not None:
        d_st, d_gs, d_rstd = dbg
        nc.sync.dma_start(d_st[:, :, :], stats[:, :, :])
        nc.sync.dma_start(d_gs[:, :, :], gs[:, :, :])
        nc.sync.dma_start(d_rstd[:, :, :], rstd[:, :, :])
    return a_t, b_t


def build_bass():
    nc = bacc.Bacc()
    x_d = nc.dram_tensor("x_cm", [C, T], F32R, kind="ExternalInput")
    wqkv_d = nc.dram_tensor("wqkvT", [C, 3 * C], F32R, kind="ExternalInput")
    wout_d = nc.dram_tensor("woutT", [C, C], F32R, kind="ExternalInput")
    bqkv_d = nc.dram_tensor("bqkv_cm", [128, 12], F32, kind="ExternalInput")
    bv_d = nc.dram_tensor("bv_rep", [128, C], F32, kind="ExternalInput")
    bout_d = nc.dram_tensor("bout_cm", [128, CSUB], F32, kind="ExternalInput")
    g1s_d = nc.dram_tensor("gn1s_cm", [128, CSUB], F32, kind="ExternalInput")
    g1o_d = nc.dram_tensor("gn1o_cm", [128, CSUB], F32, kind="ExternalInput")
    g2s_d = nc.dram_tensor("gn2s_cm", [128, CSUB], F32, kind="ExternalInput")
    g2o_d = nc.dram_tensor("gn2o_cm", [128, CSUB], F32, kind="ExternalInput")
    aagg_d = nc.dram_tensor("a_agg", [128, 8], F32, kind="ExternalInput")
    amap_d = nc.dram_tensor("a_map", [8, 128], F32, kind="ExternalInput")
    ones_d = nc.dram_tensor("ones64", [128, 64], FP16, kind="ExternalInput")
    out_d = nc.dram_tensor("out", [C, T], F32, kind="ExternalOutput")
    DEBUG = bool(__import__("os").environ.get("KDEBUG"))
    if DEBUG:
        dbg_xn = nc.dram_tensor("dbg_xn", [C, T], F32, kind="ExternalOutput")
        dbg_qk = nc.dram_tensor("dbg_qk", [1024, T], F32, kind="ExternalOutput")
        dbg_v = nc.dram_tensor("dbg_v", [T, C], F32, kind="ExternalOutput")
        dbg_r = nc.dram_tensor("dbg_r", [128, 4, 1024], F32, kind="ExternalOutput")
        dbg_y = nc.dram_tensor("dbg_y", [C, T], F32, kind="ExternalOutput")
        dbg_o = nc.dram_tensor("dbg_o", [C, T], F32, kind="ExternalOutput")
        dbg_st = nc.dram_tensor("dbg_st", [128, CSUB, 2], F32, kind="ExternalOutput")
        dbg_gs = nc.dram_tensor("dbg_gs", [8, CSUB, 2], F32, kind="ExternalOutput")
        dbg_rstd = nc.dram_tensor("dbg_rstd", [8, CSUB, 2], F32, kind="ExternalOutput")
        dbg_ab = nc.dram_tensor("dbg_ab", [128, CSUB, 4], F32, kind="ExternalOutput")

    with tile.TileContext(nc) as tc:
        with (
            tc.tile_pool(name="big", bufs=1) as big,
            tc.tile_pool(name="sb_small", bufs=2) as sb_small,
            tc.tile_pool(name="epool", bufs=4) as epool,
            tc.tile_pool(name="cpool", bufs=3) as cpool,
            tc.tile_pool(name="psum", bufs=1, space="PSUM") as psum,
            tc.tile_pool(name="psum2", bufs=2, space="PSUM") as psum2,
        ):
            pools = {"sb_small": sb_small, "psum": psum}

            # ---- resident tensors ----
            xcm = big.tile([128, CSUB, T], F32R, tag="xcm")
            wqkv = big.tile([128, CSUB, 3 * C], F32R, tag="wqkv")
            wout = big.tile([128, CSUB, C], F32R, tag="wout")
            bqkv = big.tile([128, 12], F32, tag="bqkv")
            bvr = big.tile([128, C], F32, tag="bvr")
            bout = big.tile([128, CSUB], F32, tag="bout")
            g1s = big.tile([128, CSUB], F32, tag="g1s")
            g1o = big.tile([128, CSUB], F32, tag="g1o")
            g2s = big.tile([128, CSUB], F32, tag="g2s")
            g2o = big.tile([128, CSUB], F32, tag="g2o")
            aagg = big.tile([128, 8], F32, tag="aagg")
            amap = big.tile([8, 128], F32, tag="amap")
            ones64 = big.tile([128, 64], FP16, tag="ones64")
            qk = big.tile([128, 8, T], F32R, tag="qk")
            vtm = big.tile([128, TSUB, C], FP16, tag="vtm")
            ycm = big.tile([128, CSUB, T], F32R, tag="ycm")
            ocm = big.tile([128, CSUB, T], F32, tag="ocm")
            ab1 = big.tile([128, CSUB, 4], F32, tag="ab1")

            dma = nc.sync
            dma.dma_start(xcm[:, :, :], x_d.rearrange("(ko kp) t -> kp ko t", kp=128))
            dma.dma_start(wqkv[:, :, :],
                          wqkv_d.rearrange("(ko kp) o -> kp ko o", kp=128))
            dma.dma_start(wout[:, :, :],
                          wout_d.rearrange("(ko kp) o -> kp ko o", kp=128))
            for t_sb, t_dr in ((bqkv, bqkv_d), (bvr, bv_d), (bout, bout_d),
                               (g1s, g1s_d), (g1o, g1o_d), (g2s, g2s_d),
                               (g2o, g2o_d), (aagg, aagg_d), (amap, amap_d),
                               (ones64, ones_d)):
                dma.dma_start(t_sb[:], t_dr[:])

            # ---- GroupNorm 1 (stats on raw x, then apply in place) ----
            _groupnorm(nc, tc, pools, xcm, g1s, g1o, aagg, amap,
                       (ab1[:, :, 0], ab1[:, :, 1]),
                       dbg=(dbg_st, dbg_gs, dbg_rstd) if DEBUG else None)
            if DEBUG:
                nc.sync.dma_start(dbg_ab[:, :, :], ab1[:, :, :])
            for i in range(CSUB):
                nc.vector.tensor_scalar(
                    out=xcm[:, i, :], in0=xcm[:, i, :],
                    scalar1=ab1[:, i, 0:1], scalar2=ab1[:, i, 1:2],
                    op0=mybir.AluOpType.mult, op1=mybir.AluOpType.add)

            if DEBUG:
                nc.sync.dma_start(dbg_xn.rearrange("(ko kp) t -> kp ko t", kp=128),
                                  xcm[:, :, :].bitcast(F32))

            # ---- q,k projections (channel-major) ----
            for oi in range(8):
                qk_ps = psum2.tile([128, 2, 512], F32, tag="sc")
                for th in range(2):
                    for ci in range(CSUB):
                        nc.tensor.matmul(
                            qk_ps[:, th, :],
                            lhsT=wqkv[:, ci, 128 * oi:128 * (oi + 1)],
                            rhs=xcm[:, ci, 512 * th:512 * (th + 1)],
                            start=(ci == 0), stop=(ci == CSUB - 1))
                nc.vector.tensor_scalar_add(qk[:, oi, :], qk_ps[:, :, :],
                                            bqkv[:, oi:oi + 1])

            if DEBUG:
                nc.sync.dma_start(dbg_qk.rearrange("(ko kp) t -> kp ko t", kp=128),
                                  qk[:, :, :].bitcast(F32))

            # ---- v projection (token-major, +bias, cast bf16) ----
            for ti in range(TSUB):
                v_ps = psum2.tile([128, 2, 512], F32, tag="sc")
                for ci in range(CSUB):
                    nc.tensor.matmul(
                        v_ps[:, 0, :],
                        lhsT=xcm[:, ci, 128 * ti:128 * (ti + 1)],
                        rhs=wqkv[:, ci, 1024:1536],
                        start=(ci == 0), stop=(ci == CSUB - 1))
                nc.vector.tensor_tensor(vtm[:, ti, :], v_ps[:, 0, :], bvr[:, :],
                                        mybir.AluOpType.add)

            if DEBUG:
                nc.gpsimd.dma_start(dbg_v.rearrange("(ko kp) t -> kp ko t", kp=128),
                                    vtm[:, :, :])

            # ---- attention per head pair ----
            shift_t = big.tile([128, 1], F32, tag="shift")
            nc.vector.memset(shift_t[:, :], -4.0)
            for p in range(4):
                d_ps = psum.tile([128, 2, 512], F32, tag="acc")
                y_ps = psum.tile([128, 2, 512], F32, tag="acc2")
                for si in range(TSUB):
                    for th in range(2):
                        sc_ps = psum2.tile([128, 2, 512], F32, tag="sc")
                        for hh in range(2):
                            nc.tensor.matmul(
                                sc_ps[:, hh, :],
                                lhsT=qk[64 * hh:64 * hh + 64, 4 + p,
                                        128 * si:128 * (si + 1)],
                                rhs=qk[64 * hh:64 * hh + 64, p,
                                       512 * th:512 * (th + 1)],
                                start=True, stop=True,
                                tile_position=(64 * hh, 0))
                        e_t = epool.tile([128, 2, 512], FP16, tag="e")
                        nc.scalar.activation(out=e_t[:, :, :], in_=sc_ps[:, :, :],
                                             func=mybir.ActivationFunctionType.Exp,
                                             bias=shift_t[:, :], scale=1.0)
                        for hh in range(2):
                            nc.tensor.matmul(
                                d_ps[64 * hh:64 * hh + 64, th, :],
                                lhsT=ones64[:, :],
                                rhs=e_t[:, hh, :],
                                start=(si == 0), stop=(si == TSUB - 1),
                                tile_position=(0, 64 * hh))
                            nc.tensor.matmul(
                                y_ps[64 * hh:64 * hh + 64, th, :],
                                lhsT=vtm[:, si, 64 * (2 * p + hh):
                                         64 * (2 * p + hh) + 64],
                                rhs=e_t[:, hh, :],
                                start=(si == 0), stop=(si == TSUB - 1),
                                tile_position=(0, 64 * hh))
                r_t = cpool.tile([128, 2, 512], F32, tag="recip")
                nc.vector.reciprocal_approx_fast(out=r_t[:, :, :], in_=d_ps[:, :, :])
                nc.vector.tensor_tensor(ycm[:, p, :], y_ps[:, :, :], r_t[:, :, :],
                                        mybir.AluOpType.mult)
                if DEBUG:
                    nc.sync.dma_start(dbg_r[:, p, :], r_t[:, :, :])

            # ---- output projection ----
            for oi in range(CSUB):
                o_ps = psum2.tile([128, 2, 512], F32, tag="sc")
                for th in range(2):
                    for ci in range(CSUB):
                        nc.tensor.matmul(
                            o_ps[:, th, :],
                            lhsT=wout[:, ci, 128 * oi:128 * (oi + 1)],
                            rhs=ycm[:, ci, 512 * th:512 * (th + 1)],
                            start=(ci == 0), stop=(ci == CSUB - 1))
                nc.vector.tensor_scalar_add(ocm[:, oi, :], o_ps[:, :, :],
                                            bout[:, oi:oi + 1])

            if DEBUG:
                nc.sync.dma_start(dbg_o.rearrange("(ko kp) t -> kp ko t", kp=128),
                                  ocm[:, :, :])

            # ---- GroupNorm 2 + residual ----
            ab2 = big.tile([128, CSUB, 4], F32, tag="ab2")
            _groupnorm(nc, tc, pools, ocm, g2s, g2o, aagg, amap,
                       (ab2[:, :, 0], ab2[:, :, 1]))
            for i in range(CSUB):
                tmp = cpool.tile([128, 1024], F32, tag="fin")
                nc.vector.tensor_scalar(
                    out=tmp[:, :], in0=ocm[:, i, :],
                    scalar1=ab2[:, i, 0:1], scalar2=ab2[:, i, 1:2],
                    op0=mybir.AluOpType.mult, op1=mybir.AluOpType.add)
                nc.vector.tensor_tensor(ocm[:, i, :], tmp[:, :], xcm[:, i, :],
                                        mybir.AluOpType.add)
                nc.sync.dma_start(
                    out_d.rearrange("(ko kp) t -> kp ko t", kp=128)[:, i, :],
                    ocm[:, i, :])
    nc.compile()
    return nc


def _host_inputs(x, gn1_scale, gn1_offset, w_qkv, b_qkv, w_out, b_out,
                 gn2_scale, gn2_offset):
    f = np.float32
    wqkvT = np.ascontiguousarray(w_qkv.astype(f).T)       # [512, 1536]
    wqkvT[:, :C] *= 0.125                                  # fold qk scale into q
    bq = np.asarray(b_qkv, f).copy()
    bq[:C] *= 0.125
    shared = {
        "wqkvT": wqkvT,
        "woutT": np.ascontiguousarray(w_out.astype(f).T),
        "bqkv_cm": np.ascontiguousarray(bq.reshape(12, 128).T),
        "bv_rep": np.ascontiguousarray(
            np.broadcast_to(np.asarray(b_qkv, f)[2 * C:], (128, C))),
        "bout_cm": np.ascontiguousarray(np.asarray(b_out, f).reshape(CSUB, 128).T),
        "gn1s_cm": np.ascontiguousarray(np.asarray(gn1_scale, f).reshape(CSUB, 128).T),
        "gn1o_cm": np.ascontiguousarray(np.asarray(gn1_offset, f).reshape(CSUB, 128).T),
        "gn2s_cm": np.ascontiguousarray(np.asarray(gn2_scale, f).reshape(CSUB, 128).T),
        "gn2o_cm": np.ascontiguousarray(np.asarray(gn2_offset, f).reshape(CSUB, 128).T),
        "a_agg": np.ascontiguousarray(
            (np.arange(128)[:, None] // GSIZE == np.arange(8)[None, :]).astype(f)),
        "a_map": np.ascontiguousarray(
            (np.arange(8)[:, None] == np.arange(128)[None, :] // GSIZE).astype(f)),
        "ones64": np.ones((128, 64), dtype=np.float16),
    }
    xs = np.asarray(x, f)
    in_maps = []
    for i in range(N_CORES):
        m = dict(shared)
        m["x_cm"] = np.ascontiguousarray(xs[i].reshape(T, C).T)
        in_maps.append(m)
    return in_maps


def run(trace=False, **inputs):
    if "nc" not in _CACHE:
        _CACHE["nc"] = build_bass()
    nc = _CACHE["nc"]
    in_maps = _host_inputs(**inputs)
    res = run_bass_kernel_spmd(nc, in_maps, core_ids=list(range(N_CORES)),
                               trace=trace)
    outs = [r["out"].T.reshape(32, 32, C) for r in res.results]
    return np.stack(outs).astype(np.float32), res


def kernel(**inputs):
    out, _ = run(trace=False, **inputs)
    return out


# revision 15
# speedup vs baseline: 1.1602x; 1.1602x over previous
"""AttentionBlock TRN2 kernel: data-parallel over batch (1 sample per core).

Everything on-chip is channel-major [c on partitions, t=h*w on free dim];
host pre-transposes x / weights and post-transposes the output.
"""

import numpy as np
import ml_dtypes

import concourse.bass as bass
from concourse import bacc
import concourse.mybir as mybir
import concourse.tile as tile
from concourse.bass_utils import run_bass_kernel_spmd

F32 = mybir.dt.float32
F32R = mybir.dt.float32r
BF16 = mybir.dt.bfloat16
FP16 = mybir.dt.float16

N_CORES = 8
T = 1024          # tokens = h*w = 32*32
C = 512           # channels
H = 8             # heads
D = 64            # head dim
G = 32            # groups
GSIZE = C // G    # 16 channels per group
EPS = 1e-5
CSUB = C // 128   # 4 channel subtiles
TSUB = T // 128   # 8 token subtiles
NELEM_GROUP = float(GSIZE)  # channel stats are already per-t means

_CACHE = {}


def _groupnorm(nc, tc, pools, xcm, gscale, goff, a_agg, a_map, out_ab, dbg=None):
    """Compute per-channel affine (a, b) implementing GroupNorm on channel-major
    xcm [128, CSUB, T].  a = rstd_g*scale_c, b = off_c - mean_g*rstd_g*scale_c.
    Writes them into out_ab ([128, CSUB] each).  Returns (a, b)."""
    sb, ps = pools["sb_small"], pools["psum"]
    a_t, b_t = out_ab

    stats = sb.tile([128, CSUB, 2], F32, tag="gn_stats")
    seg = sb.tile([128, 2, 6], F32, tag="gn_seg")
    for i in range(CSUB):
        for j in range(2):
            nc.vector.bn_stats(out=seg[:, j, :], in_=xcm[:, i, j * 512:(j + 1) * 512])
        nc.vector.bn_aggr(out=stats[:, i, :], in_=seg[:, :, :])
    # stats[:, :, 0]=mean_c, [:, :, 1]=var_c -> convert var to E[x^2]
    msq = sb.tile([128, CSUB], F32, tag="gn_msq")
    nc.vector.tensor_tensor(msq[:, :], stats[:, :, 0], stats[:, :, 0],
                            mybir.AluOpType.mult)
    nc.vector.tensor_tensor(stats[:, :, 1], stats[:, :, 1], msq[:, :],
                            mybir.AluOpType.add)
    # group aggregation: out[g_local, (i, s)] = sum over 16-ch groups
    gst_ps = ps.tile([128, 2, 512], F32, tag="acc")  # reuse accum psum slot
    gps = gst_ps[0:8, 0, 0:2 * CSUB]
    nc.tensor.matmul(gps, lhsT=a_agg[:, :], rhs=stats[:, :, :], start=True, stop=True)
    gs = sb.tile([8, CSUB, 2], F32, tag="gn_gs")
    nc.vector.tensor_scalar_mul(gs[:, :, :], gps, 1.0 / NELEM_GROUP)
    # var_g = msq_g - mean_g^2 ; rstd = exp(-0.5*ln(var+eps))
    vg = sb.tile([8, CSUB], F32, tag="gn_vg")
    nc.vector.tensor_tensor(vg[:, :], gs[:, :, 0], gs[:, :, 0], mybir.AluOpType.mult)
    nc.vector.tensor_tensor(vg[:, :], gs[:, :, 1], vg[:, :],
                            mybir.AluOpType.subtract)
    eps_t = sb.tile([8, 1], F32, tag="gn_eps")
    nc.vector.memset(eps_t[:, :], float(EPS))
    nc.scalar.activation(out=vg[:, :], in_=vg[:, :],
                         func=mybir.ActivationFunctionType.Ln, bias=eps_t[:, :],
                         scale=1.0)
    rstd = sb.tile([8, CSUB, 2], F32, tag="gn_rstd")
    nc.scalar.activation(out=rstd[:, :, 0], in_=vg[:, :],
                         func=mybir.ActivationFunctionType.Exp, scale=-0.5)
    nc.vector.tensor_tensor(rstd[:, :, 1], gs[:, :, 0], rstd[:, :, 0],
                            mybir.AluOpType.mult)
    # map back to channels: ab_ps[p, (i, s)] = (rstd_c, mean*rstd_c)
    ab_big = ps.tile([128, 2, 512], F32, tag="acc")
    ab_ps = ab_big[:, 0, 0:2 * CSUB]
    nc.tensor.matmul(ab_ps, lhsT=a_map[:, :], rhs=rstd[:, :, :], start=True, stop=True)
    ab_v = ab_ps.rearrange("p (i s) -> p i s", s=2)
    nc.vector.tensor_tensor(a_t[:, :], ab_v[:, :, 0], gscale[:, :],
                            mybir.AluOpType.mult)
    tmpb = sb.tile([128, CSUB], F32, tag="gn_tmpb")
    nc.vector.tensor_tensor(tmpb[:, :], ab_v[:, :, 1], gscale[:, :],
                            mybir.AluOpType.mult)
    nc.vector.tensor_tensor(b_t[:, :], goff[:, :], tmpb[:, :],
                            mybir.AluOpType.subtract)
    if dbg is not None:
        d_st, d_gs, d_rstd = dbg
        nc.sync.dma_start(d_st[:, :, :], stats[:, :, :])
        nc.sync.dma_start(d_gs[:, :, :], gs[:, :, :])
        nc.sync.dma_start(d_rstd[:, :, :], rstd[:, :, :])
    return a_t, b_t


def build_bass():
    nc = bacc.Bacc()
    x_d = nc.dram_tensor("x_cm", [C, T], F32R, kind="ExternalInput")
    wqkv_d = nc.dram_tensor("wqkvT", [C, 3 * C], F32R, kind="ExternalInput")
    wout_d = nc.dram_tensor("woutT", [C, C], F32R, kind="ExternalInput")
    bqkv_d = nc.dram_tensor("bqkv_cm", [128, 12], F32, kind="ExternalInput")
    bv_d = nc.dram_tensor("bv_rep", [128, C], F32, kind="ExternalInput")
    bout_d = nc.dram_tensor("bout_cm", [128, CSUB], F32, kind="ExternalInput")
    g1s_d = nc.dram_tensor("gn1s_cm", [128, CSUB], F32, kind="ExternalInput")
    g1o_d = nc.dram_tensor("gn1o_cm", [128, CSUB], F32, kind="ExternalInput")
    g2s_d = nc.dram_tensor("gn2s_cm", [128, CSUB], F32, kind="ExternalInput")
    g2o_d = nc.dram_tensor("gn2o_cm", [128, CSUB], F32, kind="ExternalInput")
    aagg_d = nc.dram_tensor("a_agg", [128, 8], F32, kind="ExternalInput")
    amap_d = nc.dram_tensor("a_map", [8, 128], F32, kind="ExternalInput")
    ones_d = nc.dram_tensor("ones64", [128, 64], FP16, kind="ExternalInput")
    out_d = nc.dram_tensor("out", [C, T], F32, kind="ExternalOutput")
    DEBUG = bool(__import__("os").environ.get("KDEBUG"))
    if DEBUG:
        dbg_xn = nc.dram_tensor("dbg_xn", [C, T], F32, kind="ExternalOutput")
        dbg_qk = nc.dram_tensor("dbg_qk", [1024, T], F32, kind="ExternalOutput")
        dbg_v = nc.dram_tensor("dbg_v", [T, C], F32, kind="ExternalOutput")
        dbg_r = nc.dram_tensor("dbg_r", [128, 4, 1024], F32, kind="ExternalOutput")
        dbg_y = nc.dram_tensor("dbg_y", [C, T], F32, kind="ExternalOutput")
        dbg_o = nc.dram_tensor("dbg_o", [C, T], F32, kind="ExternalOutput")
        dbg_st = nc.dram_tensor("dbg_st", [128, CSUB, 2], F32, kind="ExternalOutput")
        dbg_gs = nc.dram_tensor("dbg_gs", [8, CSUB, 2], F32, kind="ExternalOutput")
        dbg_rstd = nc.dram_tensor("dbg_rstd", [8, CSUB, 2], F32, kind="ExternalOutput")
        dbg_ab = nc.dram_tensor("dbg_ab", [128, CSUB, 4], F32, kind="ExternalOutput")

    with tile.TileContext(nc) as tc:
        with (
            tc.tile_pool(name="big", bufs=1) as big,
            tc.tile_pool(name="sb_small", bufs=2) as sb_small,
            tc.tile_pool(name="epool", bufs=4) as epool,
            tc.tile_pool(name="cpool", bufs=3) as cpool,
            tc.tile_pool(name="psum", bufs=1, space="PSUM") as psum,
            tc.tile_pool(name="psum2", bufs=2, space="PSUM") as psum2,
        ):
            pools = {"sb_small": sb_small, "psum": psum}

            # ---- resident tensors ----
            xcm = big.tile([128, CSUB, T], F32R, tag="xcm")
            wqkv = big.tile([128, CSUB, 3 * C], F32R, tag="wqkv")
            wout = big.tile([128, CSUB, C], F32R, tag="wout")
            bqkv = big.tile([128, 12], F32, tag="bqkv")
            bvr = big.tile([128, C], F32, tag="bvr")
            bout = big.tile([128, CSUB], F32, tag="bout")
            g1s = big.tile([128, CSUB], F32, tag="g1s")
            g1o = big.tile([128, CSUB], F32, tag="g1o")
            g2s = big.tile([128, CSUB], F32, tag="g2s")
            g2o = big.tile([128, CSUB], F32, tag="g2o")
            aagg = big.tile([128, 8], F32, tag="aagg")
            amap = big.tile([8, 128], F32, tag="amap")
            ones64 = big.tile([128, 64], FP16, tag="ones64")
            qk = big.tile([128, 8, T], F32R, tag="qk")
            vtm = big.tile([128, TSUB, C], FP16, tag="vtm")
            ycm = big.tile([128, CSUB, T], F32R, tag="ycm")
            ocm = big.tile([128, CSUB, T], F32, tag="ocm")
            ab1 = big.tile([128, CSUB, 4], F32, tag="ab1")

            dma = nc.sync
            dma.dma_start(xcm[:, :, :], x_d.rearrange("(ko kp) t -> kp ko t", kp=128))
            dma.dma_start(wqkv[:, :, :],
                          wqkv_d.rearrange("(ko kp) o -> kp ko o", kp=128))
            dma.dma_start(wout[:, :, :],
                          wout_d.rearrange("(ko kp) o -> kp ko o", kp=128))
            for t_sb, t_dr in ((bqkv, bqkv_d), (bvr, bv_d), (bout, bout_d),
                               (g1s, g1s_d), (g1o, g1o_d), (g2s, g2s_d),
                               (g2o, g2o_d), (aagg, aagg_d), (amap, amap_d),
                               (ones64, ones_d)):
                dma.dma_start(t_sb[:], t_dr[:])

            # ---- GroupNorm 1 (stats on raw x, then apply in place) ----
            _groupnorm(nc, tc, pools, xcm, g1s, g1o, aagg, amap,
                       (ab1[:, :, 0], ab1[:, :, 1]),
                       dbg=(dbg_st, dbg_gs, dbg_rstd) if DEBUG else None)
            if DEBUG:
                nc.sync.dma_start(dbg_ab[:, :, :], ab1[:, :, :])
            for i in range(CSUB):
                nc.vector.tensor_scalar(
                    out=xcm[:, i, :], in0=xcm[:, i, :],
                    scalar1=ab1[:, i, 0:1], scalar2=ab1[:, i, 1:2],
                    op0=mybir.AluOpType.mult, op1=mybir.AluOpType.add)

            if DEBUG:
                nc.sync.dma_start(dbg_xn.rearrange("(ko kp) t -> kp ko t", kp=128),
                                  xcm[:, :, :].bitcast(F32))

            # ---- q,k projections (channel-major) ----
            for oi in range(8):
                qk_ps = psum2.tile([128, 2, 512], F32, tag="sc")
                for th in range(2):
                    for ci in range(CSUB):
                        nc.tensor.matmul(
                            qk_ps[:, th, :],
                            lhsT=wqkv[:, ci, 128 * oi:128 * (oi + 1)],
                            rhs=xcm[:, ci, 512 * th:512 * (th + 1)],
                            start=(ci == 0), stop=(ci == CSUB - 1))
                nc.vector.tensor_scalar_add(qk[:, oi, :], qk_ps[:, :, :],
                                            bqkv[:, oi:oi + 1])

            if DEBUG:
                nc.sync.dma_start(dbg_qk.rearrange("(ko kp) t -> kp ko t", kp=128),
                                  qk[:, :, :].bitcast(F32))

            # ---- v projection (token-major, +bias, cast bf16) ----
            for ti in range(TSUB):
                v_ps = psum2.tile([128, 2, 512], F32, tag="sc")
                for ci in range(CSUB):
                    nc.tensor.matmul(
                        v_ps[:, 0, :],
                        lhsT=xcm[:, ci, 128 * ti:128 * (ti + 1)],
                        rhs=wqkv[:, ci, 1024:1536],
                        start=(ci == 0), stop=(ci == CSUB - 1))
                nc.vector.tensor_tensor(vtm[:, ti, :], v_ps[:, 0, :], bvr[:, :],
                                        mybir.AluOpType.add)

            if DEBUG:
                nc.gpsimd.dma_start(dbg_v.rearrange("(ko kp) t -> kp ko t", kp=128),
                                    vtm[:, :, :])

            # ---- attention per head pair ----
            shift_t = big.tile([128, 1], F32, tag="shift")
            nc.vector.memset(shift_t[:, :], -4.0)
            for p in range(4):
                d_ps = psum.tile([128, 2, 512], F32, tag="acc")
                y_ps = psum.tile([128, 2, 512], F32, tag="acc2")
                # software pipeline: issue scores[k]+exp[k] before D/av[k-1]
                # so the in-order PE queue never stalls waiting for ACT.
                from collections import deque
                pending = deque()  # (e_t, si, th) awaiting D/av
                for si in range(TSUB):
                    for th in range(2):
                        sc_ps = psum2.tile([128, 2, 512], F32, tag="sc")
                        for hh in range(2):
                            nc.tensor.matmul(
                                sc_ps[:, hh, :],
                                lhsT=qk[64 * hh:64 * hh + 64, 4 + p,
                                        128 * si:128 * (si + 1)],
                                rhs=qk[64 * hh:64 * hh + 64, p,
                                       512 * th:512 * (th + 1)],
                                start=True, stop=True,
                                tile_position=(64 * hh, 0))
                        e_t = epool.tile([128, 2, 512], FP16, tag="e")
                        nc.scalar.activation(out=e_t[:, :, :], in_=sc_ps[:, :, :],
                                             func=mybir.ActivationFunctionType.Exp,
                                             bias=shift_t[:, :], scale=1.0)
                        if len(pending) >= 2:
                            pe_t, psi, pth = pending.popleft()
                            for hh in range(2):
                                nc.tensor.matmul(
                                    d_ps[64 * hh:64 * hh + 64, pth, :],
                                    lhsT=ones64[:, :],
                                    rhs=pe_t[:, hh, :],
                                    start=(psi == 0), stop=(psi == TSUB - 1),
                                    tile_position=(0, 64 * hh))
                                nc.tensor.matmul(
                                    y_ps[64 * hh:64 * hh + 64, pth, :],
                                    lhsT=vtm[:, psi, 64 * (2 * p + hh):
                                             64 * (2 * p + hh) + 64],
                                    rhs=pe_t[:, hh, :],
                                    start=(psi == 0), stop=(psi == TSUB - 1),
                                    tile_position=(0, 64 * hh))
                        pending.append((e_t, si, th))
                while pending:
                    pe_t, psi, pth = pending.popleft()
                    for hh in range(2):
                        nc.tensor.matmul(
                            d_ps[64 * hh:64 * hh + 64, pth, :],
                            lhsT=ones64[:, :],
                            rhs=pe_t[:, hh, :],
                            start=(psi == 0), stop=(psi == TSUB - 1),
                            tile_position=(0, 64 * hh))
                        nc.tensor.matmul(
                            y_ps[64 * hh:64 * hh + 64, pth, :],
                            lhsT=vtm[:, psi, 64 * (2 * p + hh):
                                     64 * (2 * p + hh) + 64],
                            rhs=pe_t[:, hh, :],
                            start=(psi == 0), stop=(psi == TSUB - 1),
                            tile_position=(0, 64 * hh))
                r_t = cpool.tile([128, 2, 512], F32, tag="recip")
                nc.vector.reciprocal_approx_fast(out=r_t[:, :, :], in_=d_ps[:, :, :])
                nc.vector.tensor_tensor(ycm[:, p, :], y_ps[:, :, :], r_t[:, :, :],
                                        mybir.AluOpType.mult)
                if DEBUG:
                    nc.sync.dma_start(dbg_r[:, p, :], r_t[:, :, :])

            # ---- output projection ----
            for oi in range(CSUB):
                o_ps = psum2.tile([128, 2, 512], F32, tag="sc")
                for th in range(2):
                    for ci in range(CSUB):
                        nc.tensor.matmul(
                            o_ps[:, th, :],
                            lhsT=wout[:, ci, 128 * oi:128 * (oi + 1)],
                            rhs=ycm[:, ci, 512 * th:512 * (th + 1)],
                            start=(ci == 0), stop=(ci == CSUB - 1))
                nc.vector.tensor_scalar_add(ocm[:, oi, :], o_ps[:, :, :],
                                            bout[:, oi:oi + 1])

            if DEBUG:
                nc.sync.dma_start(dbg_o.rearrange("(ko kp) t -> kp ko t", kp=128),
                                  ocm[:, :, :])

            # ---- GroupNorm 2 + residual ----
            ab2 = big.tile([128, CSUB, 4], F32, tag="ab2")
            _groupnorm(nc, tc, pools, ocm, g2s, g2o, aagg, amap,
                       (ab2[:, :, 0], ab2[:, :, 1]))
            for i in range(CSUB):
                tmp = cpool.tile([128, 1024], F32, tag="fin")
                nc.vector.tensor_scalar(
                    out=tmp[:, :], in0=ocm[:, i, :],
                    scalar1=ab2[:, i, 0:1], scalar2=ab2[:, i, 1:2],
                    op0=mybir.AluOpType.mult, op1=mybir.AluOpType.add)
                nc.vector.tensor_tensor(ocm[:, i, :], tmp[:, :], xcm[:, i, :],
                                        mybir.AluOpType.add)
                nc.sync.dma_start(
                    out_d.rearrange("(ko kp) t -> kp ko t", kp=128)[:, i, :],
                    ocm[:, i, :])
    nc.compile()
    return nc


def _host_inputs(x, gn1_scale, gn1_offset, w_qkv, b_qkv, w_out, b_out,
                 gn2_scale, gn2_offset):
    f = np.float32
    wqkvT = np.ascontiguousarray(w_qkv.astype(f).T)       # [512, 1536]
    wqkvT[:, :C] *= 0.125                                  # fold qk scale into q
    bq = np.asarray(b_qkv, f).copy()
    bq[:C] *= 0.125
    shared = {
        "wqkvT": wqkvT,
        "woutT": np.ascontiguousarray(w_out.astype(f).T),
        "bqkv_cm": np.ascontiguousarray(bq.reshape(12, 128).T),
        "bv_rep": np.ascontiguousarray(
            np.broadcast_to(np.asarray(b_qkv, f)[2 * C:], (128, C))),
        "bout_cm": np.ascontiguousarray(np.asarray(b_out, f).reshape(CSUB, 128).T),
        "gn1s_cm": np.ascontiguousarray(np.asarray(gn1_scale, f).reshape(CSUB, 128).T),
        "gn1o_cm": np.ascontiguousarray(np.asarray(gn1_offset, f).reshape(CSUB, 128).T),
        "gn2s_cm": np.ascontiguousarray(np.asarray(gn2_scale, f).reshape(CSUB, 128).T),
        "gn2o_cm": np.ascontiguousarray(np.asarray(gn2_offset, f).reshape(CSUB, 128).T),
        "a_agg": np.ascontiguousarray(
            (np.arange(128)[:, None] // GSIZE == np.arange(8)[None, :]).astype(f)),
        "a_map": np.ascontiguousarray(
            (np.arange(8)[:, None] == np.arange(128)[None, :] // GSIZE).astype(f)),
        "ones64": np.ones((128, 64), dtype=np.float16),
    }
    xs = np.asarray(x, f)
    in_maps = []
    for i in range(N_CORES):
        m = dict(shared)
        m["x_cm"] = np.ascontiguousarray(xs[i].reshape(T, C).T)
        in_maps.append(m)
    return in_maps


def run(trace=False, **inputs):
    if "nc" not in _CACHE:
        _CACHE["nc"] = build_bass()
    nc = _CACHE["nc"]
    in_maps = _host_inputs(**inputs)
    res = run_bass_kernel_spmd(nc, in_maps, core_ids=list(range(N_CORES)),
                               trace=trace)
    outs = [r["out"].T.reshape(32, 32, C) for r in res.results]
    return np.stack(outs).astype(np.float32), res


def kernel(**inputs):
    out, _ = run(trace=False, **inputs)
    return out
